# revision 1
# baseline (speedup 1.0000x reference)
"""Trainium2 Bass kernel for nn_DGM_d (retrieval_knn).

Computes, per graph b:
    h = x[b] @ W                                  [N, Fout]
    D = cdist(h, h)^2  (clamped >= 0)
    lq = D * exp(clip(temp,-5,5)) - log(-log(q + 1e-8))
    logprobs, indices = top_k(-lq, k=16)
    edges_hat = per-graph offset edge list        (assembled host-side from indices)

Sharding: data-parallel over B=16 graphs -> 2 graphs per NeuronCore, 8 cores.
Each core runs an identical NEFF on its 2-graph slice (SPMD).

Device algorithm (per graph, per 128-row strip):
    PE:  psum_z[i,j] = 2t*G[i,j] - t*sq[j]   via one augmented matmul
         (lhsT = [hT; ones], rhs = [2t*hT; -t*sq])
    ACT: l = Ln(q + eps); t2 = Ln(-l)
    DVE: key = t2 + psum_z        (row-wise ranking key: key - t*sq_i = -lq)
    DVE: top-16 of key per row via max / max_index / match_replace (2 rounds of 8)
    DVE: logprobs = cv - t*sq_i ; slot 0 (the diagonal, always rank-1) = t2_diag
"""

import os
import numpy as np

B, N, FIN, FOUT, K = 16, 2048, 64, 32, 16
NCORES = 8
GPC = B // NCORES          # graphs per core
P = 128                    # partitions
NSTRIP = N // P            # strips per graph
EPS = 1e-8
NEG_BIG = -3.0e38

_cache = {}


def _build(t: float):
    import concourse.bacc as bacc
    import concourse.mybir as mybir
    from concourse.tile import TileContext
    from concourse.masks import make_identity

    f32 = mybir.dt.float32
    u32 = mybir.dt.uint32
    AF = mybir.ActivationFunctionType

    nc = bacc.Bacc("TRN2", target_bir_lowering=False)

    x_in = nc.declare_dram_parameter("x_in", [GPC, N, FIN], f32, isOutput=False)
    q_in = nc.declare_dram_parameter("q_in", [GPC, N, N], f32, isOutput=False)
    w_in = nc.declare_dram_parameter("w_in", [FIN, FOUT], f32, isOutput=False)
    h_out = nc.declare_dram_parameter("h_out", [GPC, N, FOUT], f32, isOutput=True)
    lp_out = nc.declare_dram_parameter("lp_out", [GPC, N, K], f32, isOutput=True)
    idx_out = nc.declare_dram_parameter("idx_out", [GPC, N, K], u32, isOutput=True)

    with TileContext(nc) as tc:
        with (
            tc.tile_pool(name="const", bufs=1) as constp,
            tc.tile_pool(name="graph", bufs=2) as gp,
            tc.tile_pool(name="strip", bufs=2) as sp,
            tc.tile_pool(name="strip1", bufs=1) as sp1,
            tc.tile_pool(name="outs", bufs=3) as op,
            tc.tile_pool(name="psum", bufs=1, space="PSUM") as pp,
        ):
            eps_col = constp.tile([P, 1], f32, tag="epsc")
            nc.vector.memset(eps_col[:], EPS)
            ones32 = constp.tile([FOUT, 1], f32, tag="ones32")
            nc.vector.memset(ones32[:], 1.0)
            ident = constp.tile([P, P], f32, tag="ident")
            make_identity(nc, ident)
            w_sb = constp.tile([FIN, FOUT], f32, tag="wsb")
            nc.sync.dma_start(w_sb[:], w_in[:])

            for g in range(GPC):
                # ---------------- per-graph preprocessing ----------------
                x_sb = gp.tile([P, NSTRIP, FIN], f32, tag="x_sb")
                nc.sync.dma_start(
                    x_sb[:], x_in[g].rearrange("(s p) f -> p s f", p=P)
                )

                # xT[f, n] via 16 PE transposes
                psum_xt = pp.tile([FIN, N], f32, tag="big")
                for s in range(NSTRIP):
                    nc.tensor.transpose(
                        psum_xt[:, s * P : (s + 1) * P], x_sb[:, s, :], ident[:]
                    )
                xT = gp.tile([FIN, N], f32, tag="xT")
                nc.scalar.copy(xT[:], psum_xt[:])

                # hT = W.T @ xT   [FOUT, N]
                psum_ht = pp.tile([FOUT, N], f32, tag="big")
                for c in range(4):
                    nc.tensor.matmul(
                        psum_ht[:, c * 512 : (c + 1) * 512],
                        w_sb[:],
                        xT[:, c * 512 : (c + 1) * 512],
                        start=True,
                        stop=True,
                    )
                htaug = gp.tile([FOUT + 1, N], f32, tag="htaug")
                nc.scalar.copy(htaug[0:FOUT, :], psum_ht[:])
                nc.vector.memset(htaug[FOUT : FOUT + 1, :], 1.0)

                # h output = transpose(hT) back to [N, FOUT]
                psum_h = pp.tile([P, NSTRIP * FOUT], f32, tag="small")
                for s in range(NSTRIP):
                    nc.tensor.transpose(
                        psum_h[:, s * FOUT : (s + 1) * FOUT],
                        htaug[0:FOUT, s * P : (s + 1) * P],
                        ident[0:FOUT, 0:FOUT],
                    )
                h_sb = gp.tile([P, NSTRIP, FOUT], f32, tag="h_sb")
                nc.scalar.copy(h_sb[:].rearrange("p s f -> p (s f)"), psum_h[:])
                nc.sync.dma_start(
                    h_out[g].rearrange("(s p) f -> p s f", p=P), h_sb[:]
                )

                # hT^2 and row sums sq
                ht2 = gp.tile([FOUT, N], f32, tag="ht2")
                nc.scalar.square(ht2[:], htaug[0:FOUT, :])

                rhs_aug = gp.tile([FOUT + 1, N], f32, tag="rhs_aug")
                nc.scalar.mul(rhs_aug[0:FOUT, :], htaug[0:FOUT, :], 2.0 * t)
                psum_sqr = pp.tile([1, N], f32, tag="big")
                for c in range(4):
                    nc.tensor.matmul(
                        psum_sqr[:, c * 512 : (c + 1) * 512],
                        ones32[:],
                        ht2[:, c * 512 : (c + 1) * 512],
                        start=True,
                        stop=True,
                    )
                nc.scalar.mul(rhs_aug[FOUT : FOUT + 1, :], psum_sqr[:], -t)

                # t*sq laid out [p, s] for per-row bias
                psum_sqc = pp.tile([P, NSTRIP], f32, tag="small")
                for s in range(NSTRIP):
                    nc.tensor.matmul(
                        psum_sqc[:, s : s + 1],
                        ht2[:, s * P : (s + 1) * P],
                        ones32[:],
                        start=True,
                        stop=True,
                    )
                tsq = gp.tile([P, NSTRIP], f32, tag="tsq")
                nc.scalar.mul(tsq[:], psum_sqc[:], t)

                # diagonal of q -> t2diag[p, s]
                qdiag = gp.tile([P, NSTRIP], f32, tag="qdiag")
                diag_ap = (
                    q_in[g]
                    .rearrange("a b -> (a b)")[0 : N * N : N + 1]
                    .rearrange("(s p) -> p s", p=P)
                )
                nc.sync.dma_start(qdiag[:], diag_ap)
                ldiag = gp.tile([P, NSTRIP], f32, tag="ldiag")
                nc.scalar.activation(ldiag[:], qdiag[:], AF.Ln, bias=eps_col[:])
                t2diag = gp.tile([P, NSTRIP], f32, tag="t2diag")
                nc.scalar.activation(t2diag[:], ldiag[:], AF.Ln, scale=-1.0)

                # ---------------- strips ----------------
                for s in range(NSTRIP):
                    qs = sp.tile([P, N], f32, tag="qs")
                    nc.sync.dma_start(qs[:], q_in[g, s * P : (s + 1) * P, :])

                    l_t = sp.tile([P, N], f32, tag="l_t")
                    nc.scalar.activation(l_t[:], qs[:], AF.Ln, bias=eps_col[:])
                    t2_t = sp.tile([P, N], f32, tag="t2_t")
                    nc.scalar.activation(t2_t[:], l_t[:], AF.Ln, scale=-1.0)

                    psum_z = pp.tile([P, N], f32, tag="big")
                    for c in range(4):
                        nc.tensor.matmul(
                            psum_z[:, c * 512 : (c + 1) * 512],
                            htaug[:, s * P : (s + 1) * P],
                            rhs_aug[:, c * 512 : (c + 1) * 512],
                            start=True,
                            stop=True,
                        )

                    key = sp1.tile([P, N], f32, tag="key")
                    nc.vector.tensor_add(key[:], t2_t[:], psum_z[:])

                    cv = op.tile([P, K], f32, tag="cv")
                    ci = op.tile([P, K], u32, tag="ci")
                    key2 = sp1.tile([P, N], f32, tag="key2")
                    nc.vector.max(out=cv[:, 0:8], in_=key[:])
                    nc.vector.max_index(out=ci[:, 0:8], in_max=cv[:, 0:8], in_values=key[:])
                    nc.vector.match_replace(
                        out=key2[:], in_to_replace=cv[:, 0:8], in_values=key[:],
                        imm_value=NEG_BIG,
                    )
                    nc.vector.max(out=cv[:, 8:16], in_=key2[:])
                    nc.vector.max_index(out=ci[:, 8:16], in_max=cv[:, 8:16], in_values=key2[:])

                    lp = op.tile([P, K], f32, tag="lp")
                    nc.vector.tensor_scalar(
                        lp[:], cv[:], tsq[:, s : s + 1], None,
                        op0=mybir.AluOpType.subtract,
                    )
                    nc.vector.tensor_copy(lp[:, 0:1], t2diag[:, s : s + 1])

                    nc.sync.dma_start(lp_out[g, s * P : (s + 1) * P, :], lp[:])
                    nc.sync.dma_start(idx_out[g, s * P : (s + 1) * P, :], ci[:])

    nc.compile()
    return nc


def _get_nc(t: float):
    key = round(float(t), 9)
    if key not in _cache:
        _cache[key] = _build(float(t))
    return _cache[key]


def kernel(x, W, temperature, q, k):
    x = np.asarray(x, dtype=np.float32)
    W = np.asarray(W, dtype=np.float32)
    q = np.asarray(q, dtype=np.float32)
    temperature = np.asarray(temperature, dtype=np.float32)
    assert int(k) == K, f"kernel compiled for k={K}, got {k}"
    assert x.shape == (B, N, FIN) and q.shape == (B, N, N) and W.shape == (FIN, FOUT)

    t = float(np.exp(np.clip(temperature[0], -5.0, 5.0)))
    nc = _get_nc(t)

    from concourse.bass_utils import run_bass_kernel_spmd

    in_maps = []
    for c in range(NCORES):
        sl = slice(c * GPC, (c + 1) * GPC)
        in_maps.append({"x_in": x[sl], "q_in": q[sl], "w_in": W})

    trace = bool(int(os.environ.get("DGM_TRACE", "0")))
    res = run_bass_kernel_spmd(
        nc, in_maps, list(range(NCORES)), trace=trace,
        trace_cores=list(range(NCORES)) if trace else None,
    )
    kernel.last_result = res

    h = np.concatenate([r["h_out"] for r in res.results], axis=0)
    logprobs = np.concatenate([r["lp_out"] for r in res.results], axis=0)
    indices = np.concatenate(
        [r["idx_out"].astype(np.int64) for r in res.results], axis=0
    )

    rows = np.broadcast_to(np.arange(N, dtype=np.int64)[None, :, None], (B, N, K))
    offs = (np.arange(B, dtype=np.int64) * N)[:, None, None]
    e0 = (indices + offs).reshape(-1)
    e1 = (rows + offs).reshape(-1)
    edges_hat = np.stack([e0, e1]).astype(np.int32)
    return h, edges_hat, logprobs
